# revision 29
# baseline (speedup 1.0000x reference)
"""Multi-head attention (B=2, S=2048, D=1024, H=16) on 8 TRN2 NeuronCores.

Sharding: core c -> (batch b = c//4, head-group g = c%4). Each core computes
the attention output restricted to its batch and its 4 heads (a 256-wide
slice of the model dim), including the row-parallel output projection
partial product (both 128-dim halves fused into one accumulation). Host
sums the 4 partials per batch and adds bo.

Device-side layouts (everything transposed so no on-device transposes are
needed):
  xq/xk/xv  bf16 [1024, 2048]  = x[b].T
  wq/wk/wv  bf16 [1024, 256]   = W[g-slice, :].T
  bqk       f32  [128, 4]      per-partition bias cols (q0,q1,k0,k1 by dch)
  bT        bf16 [1, 768]      bias row (only the V part is used on device)
  wo        bf16 [256, 1024]   = Wo[:, g-slice].T
  outT      bf16 [1024, 2048]  = (Wo_g @ ctxn_g^T) partial, host transposes

Pipeline (ACT exp is the critical engine at ~133us; everything is built
to keep it saturated):
  - 8 attention passes = (head-pair 0|1) x (4 q-windows of 512). One
    global stream of 128 chunk-steps; no pass-boundary pipeline drains.
  - scores^T[k,q]: per chunk ONE [128,1024] psum = both heads, emitted as
    two row-tiled K=64 matmuls (partitions 0-63 / 64-127) that execute
    concurrently in the PE array; ONE fd=1024 exp eviction per chunk.
  - PV: V_aug (trailing ones column -> softmax denominator Z) stationary,
    ctx[65, 512] psum accumulated over 16 chunks, lagging the exp stream.
  - spare PE slots run "grains": V projection, deferred dch1 Q/K
    projections (x tiles stay resident; no re-DMA), and the fused
    2x128-contraction output projection.
"""

import numpy as np
import ml_dtypes
from collections import deque

from concourse import bacc, tile, mybir
from concourse.bass_utils import run_bass_kernel_spmd

BF16 = mybir.dt.bfloat16
F32 = mybir.dt.float32

S = 2048      # sequence length
D = 1024      # model dim
DG = 256      # per-core head-group width (4 heads x 64)
DK = 64       # head dim
NH = 4        # heads per core
MT = 8        # model-dim contraction tiles (1024 / 128)
KC = 16       # k chunks of 128
QW = 512      # q window width
NPASS = 8     # (2 head pairs) x (4 q windows)
LAG = 2       # PV lag in chunk-steps (spool recycle pace)
N_CORES = 8
_ABLATE = set()  # timing-ablation switches; empty for the real kernel


def _emit(nc, pools, dram):
    (persist, xp, wp, wop, attnp, outp, zp, spool, cpool, gpool) = pools
    xq, xk, xv, wq, wk, wv, bqk, bT, wo, outT = dram
    Exp = mybir.ActivationFunctionType.Exp

    # ---------------- persistent tiles ----------------
    qt = [persist.tile([128, S], BF16, tag=f"qt{i}", name=f"qt{i}") for i in range(2)]
    kt = [persist.tile([128, S], BF16, tag=f"kt{i}", name=f"kt{i}") for i in range(2)]
    ctxn = [persist.tile([128, S], BF16, tag=f"ctxn{i}", name=f"ctxn{i}")
            for i in range(2)]
    vaug = persist.tile([128, KC, NH, DK + 1], BF16, tag="vaug", name="vaug")
    ones = persist.tile([1, S], BF16, tag="ones", name="ones")
    bqkt = persist.tile([128, 4], F32, tag="bqkt", name="bqkt")
    bt = persist.tile([1, 3 * DG], BF16, tag="bt", name="bt")

    nc.vector.memset(ones[:], 1.0)
    # ones column of V_aug (softmax denominator accumulates here)
    nc.vector.memset(vaug[:, :, :, DK:DK + 1], 1.0)

    # ---------------- DMA emission (consolidated 3D-AP transfers) ------
    # One DMA per x-window / weight matrix (source views via rearrange):
    # ~16 input DMAs total. SWDGE (gpsimd) generation is ~1us/DMA, so few
    # big transfers beat many per-m-tile ones.
    wq_all = wp.tile([128, MT, DG], BF16, tag="wq", name="wq")
    wk_all = wp.tile([128, MT, DG], BF16, tag="wk", name="wk")
    wv_all = wp.tile([128, MT, DG], BF16, tag="wv", name="wv")
    xk_all = xp.tile([128, MT, S], BF16, tag="xk", name="xk")
    xq_all = xp.tile([128, MT, S], BF16, tag="xq", name="xq")
    xv_all = xp.tile([128, MT, S], BF16, tag="xv", name="xv")

    def dma_xw(eng, dst, dr, lo, width):
        eng.dma_start(dst[:, :, lo:lo + width],
                      dr[:, lo:lo + width].rearrange("(m p) j -> p m j", m=MT))

    nc.gpsimd.dma_start(bqkt[:], bqk[:])
    nc.gpsimd.dma_start(bt[:], bT[:])
    dma_xw(nc.gpsimd, wq_all, wq, 0, DG)
    dma_xw(nc.gpsimd, wk_all, wk, 0, DG)
    dma_xw(nc.gpsimd, wv_all, wv, 0, DG)
    dma_xw(nc.sync, xk_all, xk, 0, 512)         # k window 0 (gate: scores)
    dma_xw(nc.sync, xq_all, xq, 0, QW)          # q window 0 (gate: Q proj)
    dma_xw(nc.scalar, xk_all, xk, 512, 512)     # k window 1 (scalar queue is input-free until the output phase)
    dma_xw(nc.scalar, xk_all, xk, 1024, 512)    # k window 2
    dma_xw(nc.sync, xv_all, xv, 0, 1024)        # xv half 0 (gate: V grains)
    dma_xw(nc.gpsimd, xk_all, xk, 1536, 512)    # k window 3
    dma_xw(nc.sync, xq_all, xq, QW, QW)         # q window 1
    dma_xw(nc.gpsimd, xv_all, xv, 1024, 1024)   # xv half 1
    dma_xw(nc.sync, xq_all, xq, 2 * QW, QW)     # q window 2
    dma_xw(nc.sync, xq_all, xq, 3 * QW, QW)     # q window 3
    wot = []
    for dch in range(2):
        t = wop.tile([128, D], BF16, tag="wo", name="wo")
        nc.sync.dma_start(t[:], wo[dch * 128:(dch + 1) * 128, :])
        wot.append(t)

    # ---------------- projection helpers ----------------
    def kq_window(key, dch, lo, width, psum_t):
        """Project [lo, lo+width) columns of q/k (key=0/1) for dch half into
        qt/kt, bias folded into the eviction."""
        w_all = wq_all if key == 0 else wk_all
        x_all = xq_all if key == 0 else xk_all
        out_t = qt[dch] if key == 0 else kt[dch]
        bcol = key * 2 + dch
        for sub in range(0, width, 512):
            for m in range(MT):
                nc.tensor.matmul(
                    psum_t[:, sub:sub + 512],
                    w_all[:, m, dch * 128:(dch + 1) * 128],
                    x_all[:, m, lo + sub:lo + sub + 512],
                    start=(m == 0), stop=(m == MT - 1))
            nc.vector.tensor_scalar_add(
                out_t[:, lo + sub:lo + sub + 512],
                psum_t[:, sub:sub + 512],
                bqkt[:, bcol:bcol + 1])

    def v_grain(sc):
        """V projection for s-chunk sc, all 4 heads in one grain."""
        vps = gpool.tile([128, NH, DK], F32, tag="g", name="vps")
        for m in range(MT):
            nc.tensor.matmul(
                vps[:, :, :],
                xv_all[:, m, sc * 128:(sc + 1) * 128],
                wv_all[:, m, :],
                start=(m == 0), stop=False)
        nc.tensor.matmul(
            vps[:, :, :],
            ones[:, sc * 128:(sc + 1) * 128],
            bt[:, 2 * DG:3 * DG],
            start=False, stop=True)
        nc.vector.tensor_copy(vaug[:, sc, :, 0:DK], vps[:, :, :])

    def kq_grain(key, dch, w):
        ps = gpool.tile([128, 512], F32, tag="g", name="kq")
        kq_window(key, dch, w * 512, 512, ps)

    def o_grain(qw, oc):
        """Fused out-projection grain: both dch halves accumulate."""
        ops = gpool.tile([128, 512], F32, tag="g", name="ops")
        for dch in range(2):
            nc.tensor.matmul(
                ops[:], wot[dch][:, oc * 128:(oc + 1) * 128],
                ctxn[dch][:, qw * QW:(qw + 1) * QW],
                start=(dch == 0), stop=(dch == 1))
        osb = outp.tile([128, 512], BF16, tag="out", name="out")
        nc.vector.tensor_copy(osb[:], ops[:])
        # scalar-queue DMA: keeps sync free so the next For_i iteration's
        # input transfers prefetch during this iteration's output phase
        nc.scalar.dma_start(
            outT[oc * 128:(oc + 1) * 128, qw * QW:(qw + 1) * QW], osb[:])

    # ---------------- P0: work needed before the chunk stream ----------
    vg_step = {}

    def run_vg(c, step):
        v_grain(c)
        vg_step[(0, c)] = step
        vg_step[(1, c)] = step

    kq_grain(0, 0, 0)                  # Q dch0 window 0
    kq_grain(1, 0, 0)                  # K dch0 window 0

    # ---------------- grain schedule ----------------
    # thresholds are emission steps; hard deadlines: Q dch0 window w before
    # step 16*w; K/Q dch1 before step 64; vg(c) before the PV that reads it
    # (enforced via vg_step gating).
    grains = deque()
    grains.append((1, lambda: kq_grain(1, 0, 1)))    # K dch0 window 1
    grains.append((3, lambda: kq_grain(1, 0, 2)))    # K dch0 window 2
    grains.append((5, lambda: kq_grain(1, 0, 3)))    # K dch0 window 3
    for i, c in enumerate(range(8)):
        grains.append((6 + i, lambda c=c, t=6 + i: run_vg(c, t)))
    grains.append((14, lambda: kq_grain(0, 0, 1)))   # Q dch0 window 1
    grains.append((22, lambda: kq_grain(0, 0, 2)))   # Q dch0 window 2
    for i, c in enumerate(range(8, KC)):
        grains.append((24 + i, lambda c=c, t=24 + i: run_vg(c, t)))
    grains.append((32, lambda: kq_grain(0, 0, 3)))   # Q dch0 window 3
    for w in range(4):
        grains.append((33 + w, lambda w=w: kq_grain(1, 1, w)))  # K dch1
    for w in range(4):
        grains.append((38 + 2 * w, lambda w=w: kq_grain(0, 1, w)))  # Q dch1
    for qw in range(3):
        for oc in range(8):
            grains.append((84 + 16 * qw + oc, lambda qw=qw, oc=oc: o_grain(qw, oc)))
    tail_grains = [(lambda oc=oc: o_grain(3, oc)) for oc in range(8)]

    # ---------------- the chunk-step stream ----------------
    att_tiles = {}
    ctx_tiles = {}

    def emit_scores(p, c):
        pair, qw = divmod(p, 4)
        qlo = qw * QW
        scs = spool.tile([128, 1024], F32, tag="s", name="scs")
        nc.tensor.matmul(
            scs[:, 0:512],
            kt[pair][0:DK, c * 128:(c + 1) * 128],
            qt[pair][0:DK, qlo:qlo + QW],
            start=True, stop=True)
        nc.tensor.matmul(
            scs[:, 512:1024],
            kt[pair][DK:128, c * 128:(c + 1) * 128],
            qt[pair][DK:128, qlo:qlo + QW],
            start=True, stop=True)
        att = attnp.tile([128, 1024], BF16, tag="attn", name="attn")
        nc.scalar.activation(att[:], scs[:], Exp, scale=0.125)
        att_tiles[(p, c)] = att

    def finish_pass(p):
        pair, qw = divmod(p, 4)
        qlo = qw * QW
        ctxA, ctxB = ctx_tiles.pop(p)
        for hl, ctx in ((0, ctxA), (1, ctxB)):
            cp = zp.tile([DK + 1, QW], F32, tag="cp", name="cp")
            nc.vector.tensor_copy(cp[:], ctx[:])  # frees the psum slot fast
            zr = zp.tile([1, QW], F32, tag="zr", name="zr")
            nc.vector.reciprocal(zr[:], cp[DK:DK + 1, :])
            bc = zp.tile([DK, QW], F32, tag="bc", name="bc")
            nc.gpsimd.partition_broadcast(bc[:], zr[:])
            nc.vector.tensor_mul(
                ctxn[pair][hl * DK:(hl + 1) * DK, qlo:qlo + QW],
                cp[0:DK, :], bc[:])

    def emit_pv(g):
        if "nopv" in _ABLATE:
            p, c = divmod(g, KC)
            att_tiles.pop((p, c))
            return
        p, c = divmod(g, KC)
        pair = p // 4
        if c == 0:
            ctx_tiles[p] = (
                cpool.tile([DK + 1, QW], F32, tag="c", name="ctxA"),
                cpool.tile([DK + 1, QW], F32, tag="c", name="ctxB"))
        ctxA, ctxB = ctx_tiles[p]
        att = att_tiles.pop((p, c))
        nc.tensor.matmul(ctxA[:], vaug[:, c, 2 * pair, :], att[:, 0:512],
                         start=(c == 0), stop=(c == KC - 1))
        nc.tensor.matmul(ctxB[:], vaug[:, c, 2 * pair + 1, :], att[:, 512:1024],
                         start=(c == 0), stop=(c == KC - 1))
        if c == KC - 1:
            finish_pass(p)

    n_steps = NPASS * KC
    pv_next = 0

    def pv_ready(g, t):
        p, c = divmod(g, KC)
        vg = vg_step.get((p // 4, c))
        return g + LAG <= t and vg is not None and vg < t

    for t in range(n_steps):
        p, c = divmod(t, KC)
        # PV first: its att dependency is pre-satisfied at steady state, so
        # the PE streams it while scores(t) waits on the spool-recycle sem
        if pv_next < n_steps and pv_ready(pv_next, t):
            emit_pv(pv_next)
            pv_next += 1
        # late-phase grains (all input DMAs long since landed) also go ahead
        # of scores to feed the PE during the spool-recycle wait; early-phase
        # grains stay behind scores to avoid DMA head-of-line blocking
        ran_grain = False
        if t >= 48 and grains and grains[0][0] <= t:
            grains.popleft()[1]()
            ran_grain = True
        emit_scores(p, c)
        # one spare PE slot: a due grain outranks a second PV (grains feed
        # future scores; PV has psum-side slack)
        if not ran_grain and grains and grains[0][0] <= t:
            grains.popleft()[1]()
        elif pv_next < n_steps and pv_ready(pv_next, t):
            emit_pv(pv_next)
            pv_next += 1

    # ---------------- tail ----------------
    while pv_next < n_steps:
        emit_pv(pv_next)
        pv_next += 1
    while grains:
        grains.popleft()[1]()
    for g in tail_grains:
        g()


def build_nc(reps=1):
    nc = bacc.Bacc("TRN2", target_bir_lowering=False)
    dram = (
        nc.dram_tensor("xq", [D, S], BF16, kind="ExternalInput"),
        nc.dram_tensor("xk", [D, S], BF16, kind="ExternalInput"),
        nc.dram_tensor("xv", [D, S], BF16, kind="ExternalInput"),
        nc.dram_tensor("wq", [D, DG], BF16, kind="ExternalInput"),
        nc.dram_tensor("wk", [D, DG], BF16, kind="ExternalInput"),
        nc.dram_tensor("wv", [D, DG], BF16, kind="ExternalInput"),
        nc.dram_tensor("bqk", [128, 4], F32, kind="ExternalInput"),
        nc.dram_tensor("bT", [1, 3 * DG], BF16, kind="ExternalInput"),
        nc.dram_tensor("wo", [DG, D], BF16, kind="ExternalInput"),
        nc.dram_tensor("outT", [D, S], BF16, kind="ExternalOutput"),
    )

    with tile.TileContext(nc) as tc:
        with (
            tc.tile_pool(name="persist", bufs=1) as persist,
            tc.tile_pool(name="xp", bufs=1) as xp,
            tc.tile_pool(name="wp", bufs=1) as wp,
            tc.tile_pool(name="wop", bufs=2) as wop,
            tc.tile_pool(name="attnp", bufs=14) as attnp,
            tc.tile_pool(name="outp", bufs=4) as outp,
            tc.tile_pool(name="zp", bufs=2) as zp,
            tc.tile_pool(name="spool", bufs=2, space="PSUM") as spool,
            tc.tile_pool(name="cpool", bufs=2, space="PSUM") as cpool,
            tc.tile_pool(name="gpool", bufs=2, space="PSUM") as gpool,
        ):
            pools = (persist, xp, wp, wop, attnp, outp, zp, spool, cpool, gpool)
            if reps == 1:
                _emit(nc, pools, dram)
            else:
                with tc.For_i(0, reps, 1):
                    _emit(nc, pools, dram)
    nc.compile()
    return nc


def make_in_maps(query, key, value, Wq, bq, Wk, bk, Wv, bv, Wo, bo):
    bf = ml_dtypes.bfloat16
    query, key, value = (np.asarray(a, np.float32) for a in (query, key, value))
    Wq, bq, Wk, bk, Wv, bv, Wo, bo = (
        np.asarray(a, np.float32) for a in (Wq, bq, Wk, bk, Wv, bv, Wo, bo))
    in_maps = []
    for c in range(N_CORES):
        b, g = divmod(c, 4)
        sl = slice(g * DG, (g + 1) * DG)

        def xa(x):
            return np.ascontiguousarray(x[b].T).astype(bf)

        def wa(W):
            return np.ascontiguousarray(W[sl, :].T).astype(bf)

        bqk_arr = np.stack([bq[sl][:128], bq[sl][128:],
                            bk[sl][:128], bk[sl][128:]], axis=1)
        in_maps.append({
            "xq": xa(query), "xk": xa(key), "xv": xa(value),
            "wq": wa(Wq), "wk": wa(Wk), "wv": wa(Wv),
            "bqk": np.ascontiguousarray(bqk_arr, np.float32),
            "bT": np.concatenate([bq[sl], bk[sl], bv[sl]])[None, :].astype(bf),
            "wo": np.ascontiguousarray(Wo[:, sl].T).astype(bf),
        })
    return in_maps


_NC_CACHE = {}


def kernel(query, key, value, Wq, bq, Wk, bk, Wv, bv, Wo, bo):
    in_maps = make_in_maps(query, key, value, Wq, bq, Wk, bk, Wv, bv, Wo, bo)
    if 1 not in _NC_CACHE:
        _NC_CACHE[1] = build_nc(1)
    nc = _NC_CACHE[1]
    res = run_bass_kernel_spmd(nc, in_maps, core_ids=list(range(N_CORES)))
    out = np.zeros((2, S, D), np.float32)
    for c in range(N_CORES):
        b = c // 4
        out[b] += np.asarray(res.results[c]["outT"], np.float32).T
    out += np.asarray(bo, np.float32)[None, None, :]
    return out
